# revision 8
# baseline (speedup 1.0000x reference)
"""GPT (4-layer, B=4 T=1024 C=1024 NH=8) Trainium2 Bass kernel.

Sharding: 8 cores = 4 batches (DP) x 2 balanced-causal sequence shards.
Shard 0 owns token chunks [0,256)+[768,1024); shard 1 owns [256,768).
This balances causal attention work across the shard pair (each core
computes 12 kv-blocks of 128 per head: 4 for its low chunk qA, 8 for its
high chunk qB; invisible blocks are zeroed by data masks so the SPMD
program is identical on every core).

Per layer: LN1 (stats via PE ones-matmuls, rstd via ACT ln/exp, output
written directly in fp8e4 scaled x16), Q/K/V projections as fp8
DoubleRow matmuls (K=256 per instruction), K/V AllGathered (bf16)
between the shard pair as two separate collectives so attention can
start on K while V is still in flight. Gathered K/V stay SBUF-resident.
Scores accumulate 4 blocks per PSUM tile so one ACT exp call evacuates
[128,1024]; causal masks are elementwise only where needed; the
column-disable mask is folded into the softmax denominator matmul
(lhsT = column mask vector) and into zeroed V rows. 1/den comes from
ACT exp(-ln(den)) and is broadcast across partitions with a PE
ones-matmul (no DRAM round trips anywhere). Attention out -> fp8 proj
(DoubleRow) -> residual. LN2 -> MLP in bf16 (fp8 fails the accuracy
budget there) with gelu evacuated straight from PSUM by ACT.
Final LN + per-residue EinLinear head as in the reference.
"""

import os
import sys

import numpy as np

for _p in ("/opt/trn_rl_repo",):
    if _p not in sys.path and os.path.isdir(_p):
        sys.path.insert(0, _p)

import ml_dtypes  # noqa: E402

BF16 = ml_dtypes.bfloat16
F8E4 = ml_dtypes.float8_e4m3  # TRN FP8_EXP4 (max +-240)

# model dims
B, T, C, L = 4, 1024, 1024, 4
NH, HD = 8, 128
HID = 4 * C
V1 = 101  # V + 1
TD = 64  # TOTAL_DIM
NUM_NODE, F_DIM = 15, 4
D_BIAS = NUM_NODE * F_DIM  # 60
EPS = 1e-5
NCORES = 8
TOK = 512  # tokens per core (2 chunks of 256)
CH = 256  # chunk width
KSUB = C // 128  # 8
HSUB = HID // 128  # 32
NL = int(os.environ.get("KERNEL_NLAYERS", str(L)))

# fp8 scales
SA = 16.0  # LN1 output / attention-out scale
SWQ = 8192.0  # wq (after 1/sqrt(hd) fold)
SW = 512.0  # wk/wv/wp
UNS_Q = 1.0 / (SA * SWQ)
UNS_KV = 1.0 / (SA * SW)
LN16 = float(np.log(SA))

# chunk starts per shard: (qA base, qB base)
CHUNKS = {0: (0, 768), 1: (256, 512)}

_CACHED = {}


def _build_program():
    import concourse.bacc as bacc
    import concourse.bass as bass  # noqa: F401
    import concourse.mybir as mybir
    import concourse.tile as tile

    dt = mybir.dt
    AF = mybir.ActivationFunctionType
    OP = mybir.AluOpType
    PM = mybir.MatmulPerfMode

    nc = bacc.Bacc(trn_type="TRN2", num_devices=NCORES)

    # ---- I/O ----
    h0T_d = nc.dram_tensor("h0T", (C, TOK), dt.float32, kind="ExternalInput")
    maskT_d = nc.dram_tensor("maskT", (8, 128, CH), dt.bfloat16, kind="ExternalInput")
    pbT_d = nc.dram_tensor("pbT", (64, NH, 64), dt.float32, kind="ExternalInput")
    wq_d = nc.dram_tensor("wq", (L, C, C), dt.float8e4, kind="ExternalInput")
    wk_d = nc.dram_tensor("wk", (L, C, C), dt.float8e4, kind="ExternalInput")
    wv_d = nc.dram_tensor("wv", (L, C, C), dt.float8e4, kind="ExternalInput")
    wp_d = nc.dram_tensor("wp", (L, C, C), dt.float8e4, kind="ExternalInput")
    w1_d = nc.dram_tensor("w1", (L, C, HID), dt.bfloat16, kind="ExternalInput")
    w2_d = nc.dram_tensor("w2", (L, HID, C), dt.bfloat16, kind="ExternalInput")
    hwT_d = nc.dram_tensor("hwT", (TD, C, V1), dt.bfloat16, kind="ExternalInput")
    cm16_d = nc.dram_tensor("cm16", (128, 1), dt.bfloat16, kind="ExternalInput")
    cm32_d = nc.dram_tensor("cm32", (128, 1), dt.float32, kind="ExternalInput")
    out_d = nc.dram_tensor("logits", (TOK, V1), dt.float32, kind="ExternalOutput")

    RG = [[0, 1], [2, 3], [4, 5], [6, 7]]
    KVHALF = C * TOK  # elements per k (or v) shard

    with tile.TileContext(nc) as tc:
        consts = tc.alloc_tile_pool(name="consts", bufs=1)
        hpool = tc.alloc_tile_pool(name="hpool", bufs=1)
        kvpool = tc.alloc_tile_pool(name="kvpool", bufs=1)
        wpool = tc.alloc_tile_pool(name="wpool", bufs=2)
        w1pool = tc.alloc_tile_pool(name="w1pool", bufs=2)
        w2pool = tc.alloc_tile_pool(name="w2pool", bufs=2)
        apool = tc.alloc_tile_pool(name="apool", bufs=1)
        spool = tc.alloc_tile_pool(name="spool", bufs=2)
        dpool = tc.alloc_tile_pool(name="dpool", bufs=2, space="DRAM")
        ps = tc.alloc_tile_pool(name="ps", bufs=4, space="PSUM")
        psc = tc.alloc_tile_pool(name="psc", bufs=2, space="PSUM")

        # ---- constants ----
        mask_sb = consts.tile([128, 8, CH], dt.bfloat16, name="mask_sb")
        nc.sync.dma_start(mask_sb, maskT_d.rearrange("m p q -> p m q"))
        pb_sb = consts.tile([64, NH, 64], dt.float32, name="pb_sb")
        nc.sync.dma_start(pb_sb, pbT_d[:])
        ones32 = consts.tile([128, 1], dt.float32, name="ones32")
        nc.vector.memset(ones32, 1.0)
        onesrow = consts.tile([1, 128], dt.float32, name="onesrow")
        nc.vector.memset(onesrow, 1.0)
        cm16 = consts.tile([128, 1], dt.bfloat16, name="cm16")
        nc.sync.dma_start(cm16, cm16_d[:])
        cm32 = consts.tile([128, 1], dt.float32, name="cm32")
        nc.sync.dma_start(cm32, cm32_d[:])
        eps1 = consts.tile([1, 1], dt.float32, name="eps1")
        nc.vector.memset(eps1, EPS)
        bln16 = consts.tile([1, 1], dt.float32, name="bln16")
        nc.vector.memset(bln16, LN16)
        bzero = consts.tile([1, 1], dt.float32, name="bzero")
        nc.vector.memset(bzero, 0.0)

        # ---- residual stream, feature-major fp32 ----
        h_sb = hpool.tile([128, KSUB, TOK], dt.float32, name="h_sb")
        nc.sync.dma_start(h_sb, h0T_d.rearrange("(ko p) q -> p ko q", p=128))

        def layernorm(dst, fp8_scale_bias):
            """LN over C of h_sb -> dst (bf16 or fp8).

            fp8_scale_bias: extra ln-space bias added to rstd (ln(SA) for
            fp8 outputs so dst = SA * normalized, 0.0 for bf16).
            """
            sum_ps = ps.tile([128, TOK], dt.float32, tag="ps", name="sum_ps")
            ssq_ps = ps.tile([128, TOK], dt.float32, tag="ps", name="ssq_ps")
            for ct in range(KSUB):
                nc.tensor.matmul(
                    sum_ps[0:1, :], ones32, h_sb[:, ct, :],
                    start=(ct == 0), stop=(ct == KSUB - 1),
                )
            for ct in range(KSUB):
                sq = spool.tile([128, TOK], dt.float32, tag="scr32", name="sq")
                nc.scalar.square(sq, h_sb[:, ct, :])
                nc.tensor.matmul(
                    ssq_ps[0:1, :], ones32, sq,
                    start=(ct == 0), stop=(ct == KSUB - 1),
                )
            # stats on one partition: mean, var, then s1 = scale*rstd, s0 = mean*s1
            st = spool.tile([1, 4, TOK], dt.float32, tag="stat", name="st", bufs=1)
            mean = st[:, 0, :]
            var = st[:, 1, :]
            s1 = st[:, 2, :]
            s0 = st[:, 3, :]
            nc.vector.tensor_scalar_mul(mean, sum_ps[0:1, :], 1.0 / C)
            nc.vector.tensor_scalar_mul(var, ssq_ps[0:1, :], 1.0 / C)
            msq = spool.tile([1, TOK], dt.float32, tag="msq", name="msq")
            nc.vector.tensor_mul(msq, mean, mean)
            nc.vector.tensor_sub(var, var, msq)
            # rstd = exp(-0.5 * ln(var + eps)) [* SA]
            lnv = spool.tile([1, TOK], dt.float32, tag="lnv", name="lnv")
            nc.scalar.activation(lnv, var, AF.Ln, bias=eps1, scale=1.0)
            nc.scalar.activation(s1, lnv, AF.Exp, bias=fp8_scale_bias, scale=-0.5)
            nc.vector.tensor_mul(s0, mean, s1)
            # broadcast via PE: statB[p, :] = s for all p
            s1B = ps.tile([128, TOK], dt.float32, tag="ps", name="s1B")
            s0B = ps.tile([128, TOK], dt.float32, tag="ps", name="s0B")
            nc.tensor.matmul(s1B, onesrow, s1, start=True, stop=True)
            nc.tensor.matmul(s0B, onesrow, s0, start=True, stop=True)
            for ct in range(KSUB):
                tmp = spool.tile([128, TOK], dt.float32, tag="scr32", name="lntmp")
                nc.vector.tensor_mul(tmp, h_sb[:, ct, :], s1B)
                nc.vector.tensor_sub(dst[:, ct, :], tmp, s0B)

        for layer in range(NL):
            # ---------- LN1 -> fp8 (x SA) ----------
            aT8 = apool.tile([128, KSUB, TOK], dt.float8e4, tag="a8", name="aT8")
            layernorm(aT8, bln16)

            # ---------- K projection (fp8 DoubleRow) + AllGather ----------
            k_in = dpool.tile([C, TOK], dt.bfloat16, tag="kin", name="k_in")
            k_ga = dpool.tile([2, C, TOK], dt.bfloat16, tag="kga", name="k_ga")
            v_in = dpool.tile([TOK, C], dt.bfloat16, tag="vin", name="v_in")
            v_ga = dpool.tile([2, TOK, C], dt.bfloat16, tag="vga", name="v_ga")

            wk_sb = wpool.tile([128, KSUB, C], dt.float8e4, tag="wmat", name="wk_sb")
            nc.sync.dma_start(wk_sb, wk_d[layer].rearrange("(ko p) n -> p ko n", p=128))
            k_sb = apool.tile([128, NH, TOK], dt.bfloat16, tag="kv_st", name="k_sb")
            for hh in range(NH):
                pk = ps.tile([128, TOK], dt.float32, tag="ps", name="pk")
                for kk in range(KSUB // 2):
                    nc.tensor.matmul(
                        pk,
                        wk_sb[:, 2 * kk : 2 * kk + 2, hh * HD : (hh + 1) * HD],
                        aT8[:, 2 * kk : 2 * kk + 2, :],
                        start=(kk == 0), stop=(kk == KSUB // 2 - 1),
                        perf_mode=PM.DoubleRow,
                    )
                nc.vector.tensor_scalar_mul(k_sb[:, hh, :], pk, UNS_KV)
            nc.sync.dma_start(
                k_in.rearrange("(h d) t -> d h t", d=128), k_sb
            )
            nc.gpsimd.collective_compute(
                "AllGather", OP.bypass, replica_groups=RG,
                ins=[k_in.opt()], outs=[k_ga.opt()],
            )

            # ---------- V projection (fp8 DoubleRow, token-major out) ----------
            wv_sb = wpool.tile([128, KSUB, C], dt.float8e4, tag="wmat", name="wv_sb")
            nc.sync.dma_start(wv_sb, wv_d[layer].rearrange("(ko p) n -> p ko n", p=128))
            v_sb = apool.tile([128, 4, C], dt.bfloat16, tag="kv_st", name="v_sb")
            for tsub in range(4):
                for chalf in range(2):
                    pv = ps.tile([128, 512], dt.float32, tag="ps", name="pv")
                    for kk in range(KSUB // 2):
                        nc.tensor.matmul(
                            pv,
                            aT8[:, 2 * kk : 2 * kk + 2, tsub * 128 : (tsub + 1) * 128],
                            wv_sb[:, 2 * kk : 2 * kk + 2, chalf * 512 : (chalf + 1) * 512],
                            start=(kk == 0), stop=(kk == KSUB // 2 - 1),
                            perf_mode=PM.DoubleRow,
                        )
                    # zero disabled token rows (p%64==63) + unscale
                    nc.vector.tensor_scalar(
                        v_sb[:, tsub, chalf * 512 : (chalf + 1) * 512],
                        pv, cm32, UNS_KV, op0=OP.mult, op1=OP.mult,
                    )
            nc.sync.dma_start(v_in.rearrange("(ts p) c -> p ts c", p=128), v_sb)
            nc.gpsimd.collective_compute(
                "AllGather", OP.bypass, replica_groups=RG,
                ins=[v_in.opt()], outs=[v_ga.opt()],
            )

            # ---------- Q projection (fp8 DoubleRow, overlaps AllGather) ----------
            wq_sb = wpool.tile([128, KSUB, C], dt.float8e4, tag="wmat", name="wq_sb")
            nc.sync.dma_start(wq_sb, wq_d[layer].rearrange("(ko p) n -> p ko n", p=128))
            qT = apool.tile([128, NH, TOK], dt.bfloat16, tag="qT", name="qT")
            for hh in range(NH):
                pq = ps.tile([128, TOK], dt.float32, tag="ps", name="pq")
                for kk in range(KSUB // 2):
                    nc.tensor.matmul(
                        pq,
                        wq_sb[:, 2 * kk : 2 * kk + 2, hh * HD : (hh + 1) * HD],
                        aT8[:, 2 * kk : 2 * kk + 2, :],
                        start=(kk == 0), stop=(kk == KSUB // 2 - 1),
                        perf_mode=PM.DoubleRow,
                    )
                nc.vector.tensor_scalar_mul(qT[:, hh, :], pq, UNS_Q)

            # ---------- gathered K/V -> SBUF (global token order) ----------
            # global chunks: c0 = ga[0][0:256], c1 = ga[1][0:256],
            #                c2 = ga[1][256:512], c3 = ga[0][256:512]
            kg = kvpool.tile([128, NH, T], dt.bfloat16, name="kg")
            CHSRC = [(0, 0), (1, 0), (1, 256), (0, 256)]
            for gch, (r, off) in enumerate(CHSRC):
                nc.sync.dma_start(
                    kg[:, :, gch * CH : (gch + 1) * CH],
                    k_ga[r].rearrange("(h d) t -> d h t", d=128)[
                        :, :, off : off + CH
                    ],
                )
            # vg layout: [p, gblk, h, d]; token = 128*gblk + p
            vg = kvpool.tile([128, 8, NH, HD], dt.bfloat16, name="vg")
            for gch, (r, off) in enumerate(CHSRC):
                nc.sync.dma_start(
                    vg[:, 2 * gch : 2 * gch + 2, :, :],
                    v_ga[r].rearrange("(ts p) (h d) -> p ts h d", p=128, d=128)[
                        :, off // 128 : off // 128 + 2, :, :
                    ],
                )

            # ---------- attention ----------
            yT8 = apool.tile([128, NH, TOK], dt.float8e4, tag="y8", name="yT8")
            for hh in range(NH):
                den = ps.tile([1, TOK], dt.float32, tag="ps", name="den")
                py = ps.tile([128, TOK], dt.float32, tag="ps", name="py")
                state = {"first": True}

                def qchunk(qoff, gbase, ngrp, masked, last=False):
                    """One group of up to 4 kv blocks for q cols [qoff,qoff+CH)."""
                    sc = psc.tile([128, 4, CH], dt.float32, tag="sc", name="sc")
                    for i in range(ngrp):
                        g = gbase + i
                        nc.tensor.matmul(
                            sc[:, i, :],
                            kg[:, hh, g * 128 : (g + 1) * 128],
                            qT[:, hh, qoff : qoff + CH],
                            start=True, stop=True,
                        )
                    if gbase == 0 and qoff == 0:
                        # graph bias on (kv<64, q<64); zero data off shard 0
                        nc.vector.tensor_add(
                            sc[0:64, 0, 0:64], sc[0:64, 0, 0:64], pb_sb[:, hh, :]
                        )
                    e = spool.tile([128, 4, CH], dt.bfloat16, tag="e_sb", name="e_sb")
                    nc.scalar.activation(
                        e[:, 0:ngrp, :], sc[:, 0:ngrp, :], AF.Exp
                    )
                    if masked is not None:
                        nc.vector.tensor_mul(
                            e[:, 0:ngrp, :], e[:, 0:ngrp, :],
                            mask_sb[:, masked : masked + ngrp, :],
                        )
                    for i in range(ngrp):
                        g = gbase + i
                        fin = last and (i == ngrp - 1)
                        nc.tensor.matmul(
                            den[:, qoff : qoff + CH], cm16, e[:, i, :],
                            start=state["first"], stop=fin,
                        )
                        nc.tensor.matmul(
                            py[:, qoff : qoff + CH], vg[:, g, hh, :], e[:, i, :],
                            start=state["first"], stop=fin,
                        )
                        state["first"] = False

                qchunk(0, 0, 4, 0)             # qA: blocks 0..3, masks 0..3
                qchunk(CH, 0, 4, None)         # qB: blocks 0..3, always visible
                qchunk(CH, 4, 4, 4, last=True) # qB: blocks 4..7, masks 4..7
                # softmax denominator -> SA/den, broadcast, y = py * recB
                lnden = spool.tile([1, TOK], dt.float32, tag="lnv", name="lnden")
                nc.scalar.activation(lnden, den, AF.Ln)
                rec = spool.tile([1, TOK], dt.float32, tag="msq", name="rec")
                nc.scalar.activation(rec, lnden, AF.Exp, bias=bln16, scale=-1.0)
                recB = ps.tile([128, TOK], dt.float32, tag="ps", name="recB")
                nc.tensor.matmul(recB, onesrow, rec, start=True, stop=True)
                py_sb = spool.tile([128, TOK], dt.bfloat16, tag="py_sb", name="py_sb")
                nc.scalar.copy(py_sb, py)
                nc.vector.tensor_mul(yT8[:, hh, :], py_sb, recB)

            # ---------- proj (fp8 DoubleRow) + residual ----------
            wp_sb = wpool.tile([128, KSUB, C], dt.float8e4, tag="wmat", name="wp_sb")
            nc.sync.dma_start(wp_sb, wp_d[layer].rearrange("(ko p) n -> p ko n", p=128))
            for co in range(KSUB):
                pp = ps.tile([128, TOK], dt.float32, tag="ps", name="pp")
                for kk in range(KSUB // 2):
                    nc.tensor.matmul(
                        pp,
                        wp_sb[:, 2 * kk : 2 * kk + 2, co * 128 : (co + 1) * 128],
                        yT8[:, 2 * kk : 2 * kk + 2, :],
                        start=(kk == 0), stop=(kk == KSUB // 2 - 1),
                        perf_mode=PM.DoubleRow,
                    )
                pp_sb = spool.tile([128, TOK], dt.float32, tag="scr32", name="pp_sb")
                nc.scalar.mul(pp_sb, pp, UNS_KV)
                nc.vector.tensor_add(h_sb[:, co, :], h_sb[:, co, :], pp_sb)

            # ---------- LN2 -> bf16 ----------
            aT2 = apool.tile([128, KSUB, TOK], dt.bfloat16, tag="a16", name="aT2")
            layernorm(aT2, 0.0)

            # ---------- MLP (bf16) ----------
            g_sb = apool.tile([128, HSUB, TOK], dt.bfloat16, tag="g_sb", name="g_sb")
            for hb in range(8):  # 512 hidden cols at a time
                w1_sb = w1pool.tile([128, KSUB, 512], dt.bfloat16, tag="w1b", name="w1_sb")
                nc.sync.dma_start(
                    w1_sb,
                    w1_d[layer].rearrange("(ko p) n -> p ko n", p=128)[
                        :, :, hb * 512 : (hb + 1) * 512
                    ],
                )
                for hc in range(4):
                    pu = ps.tile([128, TOK], dt.float32, tag="ps", name="pu")
                    for ct in range(KSUB):
                        nc.tensor.matmul(
                            pu,
                            w1_sb[:, ct, hc * 128 : (hc + 1) * 128],
                            aT2[:, ct, :],
                            start=(ct == 0), stop=(ct == KSUB - 1),
                        )
                    nc.scalar.activation(g_sb[:, hb * 4 + hc, :], pu, AF.Gelu)

            for grp in range(2):  # 4 output c-tiles at a time (PSUM budget)
                pd = [
                    ps.tile([128, TOK], dt.float32, tag="ps", name=f"pd{i}")
                    for i in range(4)
                ]
                for jc in range(8):  # w2 chunk of 512 hidden rows
                    w2_sb = w2pool.tile([128, 4, C], dt.bfloat16, tag="w2t", name="w2_sb")
                    nc.sync.dma_start(
                        w2_sb,
                        w2_d[layer][512 * jc : 512 * (jc + 1), :].rearrange(
                            "(ks p) n -> p ks n", p=128
                        ),
                    )
                    for ks in range(4):
                        ksg = 4 * jc + ks
                        for i in range(4):
                            co = grp * 4 + i
                            nc.tensor.matmul(
                                pd[i],
                                w2_sb[:, ks, co * 128 : (co + 1) * 128],
                                g_sb[:, ksg, :],
                                start=(ksg == 0), stop=(ksg == HSUB - 1),
                            )
                for i in range(4):
                    co = grp * 4 + i
                    nc.vector.tensor_add(h_sb[:, co, :], h_sb[:, co, :], pd[i])

        # ---------- final LN + head ----------
        hfT = apool.tile([128, KSUB, TOK], dt.bfloat16, tag="a16", name="hfT")
        layernorm(hfT, 0.0)
        hfT_r = hfT.rearrange("p k (b e) -> p k e b", e=TD)  # b: 8 blocks of 64
        out_r = out_d.rearrange("(b e) v -> e b v", e=TD)
        for e in range(TD):
            hw_sb = w1pool.tile([128, KSUB, V1], dt.bfloat16, tag="hw", name="hw_sb")
            nc.sync.dma_start(hw_sb, hwT_d[e].rearrange("(ko p) n -> p ko n", p=128))
            po = ps.tile([TOK // TD, V1], dt.float32, tag="ps", name="po")
            for ct in range(KSUB):
                nc.tensor.matmul(
                    po, hfT_r[:, ct, e, :], hw_sb[:, ct, :],
                    start=(ct == 0), stop=(ct == KSUB - 1),
                )
            o_sb = spool.tile([TOK // TD, V1], dt.float32, tag="o_sb", name="o_sb")
            nc.vector.tensor_copy(o_sb, po)
            nc.sync.dma_start(out_r[e], o_sb)

        for p in (psc, ps, dpool, spool, apool, w2pool, w1pool, wpool, kvpool, hpool, consts):
            p.release()

    nc.compile()
    return nc


def _host_inputs(x, attn_bias, pos_emb, Wq, Wk, Wv, Wp, w1, w2, head_w):
    """Build per-core input maps (numpy)."""
    scale = 1.0 / np.sqrt(HD)

    def to8(w, s):
        return np.clip(np.asarray(w, np.float32) * s, -240.0, 240.0).astype(F8E4)

    wq8 = to8(np.asarray(Wq, np.float32) * scale, SWQ)
    wk8 = to8(Wk, SW)
    wv8 = to8(Wv, SW)
    wp8 = to8(Wp, SW)
    w1b = np.asarray(w1, np.float32).astype(BF16)
    w2b = np.asarray(w2, np.float32).astype(BF16)
    hwT = np.ascontiguousarray(
        np.asarray(head_w, np.float32).transpose(0, 2, 1)
    ).astype(BF16)

    # pbias (graph bias) expanded; transposed (kv, head, q), padded 60->64
    bias = np.repeat(np.repeat(np.asarray(attn_bias, np.float32), F_DIM, 1), F_DIM, 2)
    pbT = np.zeros((64, NH, 64), np.float32)
    pbT[:D_BIAS, :, :D_BIAS] = bias.transpose(2, 0, 1)  # [j, h, i]
    pbT_zero = np.zeros_like(pbT)

    h0 = np.asarray(x, np.float32) + np.asarray(pos_emb, np.float32)  # (B, T, C)

    # per-shard causal masks: m in 0..3 -> qA vs kv block m; 4..7 -> qB vs block m
    masks = {}
    for shard, (cA, cB) in CHUNKS.items():
        mk = np.zeros((8, 128, CH), np.float32)
        jq = np.arange(CH)
        for m in range(4):
            kvi = m * 128 + np.arange(128)
            mk[m] = (kvi[:, None] <= (cA + jq)[None, :])
        for m in range(4, 8):
            kvi = m * 128 + np.arange(128)
            mk[m] = (kvi[:, None] <= (cB + jq)[None, :])
        masks[shard] = mk.astype(BF16)

    cmv = np.ones((128, 1), np.float32)
    cmv[63, 0] = 0.0
    cmv[127, 0] = 0.0
    in_maps = []
    for core in range(NCORES):
        b, shard = core // 2, core % 2
        cA, cB = CHUNKS[shard]
        tok = np.r_[cA : cA + CH, cB : cB + CH]
        h0T = np.ascontiguousarray(h0[b, tok].T)  # (C, TOK)
        in_maps.append(
            {
                "h0T": h0T,
                "maskT": masks[shard],
                "pbT": pbT if shard == 0 else pbT_zero,
                "wq": wq8, "wk": wk8, "wv": wv8, "wp": wp8,
                "w1": w1b, "w2": w2b, "hwT": hwT,
                "cm16": cmv.astype(BF16), "cm32": cmv,
            }
        )
    return in_maps


def kernel(**inputs):
    from concourse.bass_utils import run_bass_kernel_spmd

    in_maps = _host_inputs(
        inputs["x"], inputs["attn_bias"], inputs["pos_emb"],
        inputs["Wq"], inputs["Wk"], inputs["Wv"], inputs["Wp"],
        inputs["w1"], inputs["w2"], inputs["head_w"],
    )
    if "nc" not in _CACHED:
        _CACHED["nc"] = _build_program()

    if os.environ.get("KERNEL_SIM", "0") == "1":
        from concourse.bass_interp import MultiCoreSim

        sim = MultiCoreSim(_CACHED["nc"], NCORES)
        for t, m in enumerate(in_maps):
            for k, v in m.items():
                sim.cores[t].tensor(k)[:] = v
        sim.simulate()
        results = [
            {"logits": np.array(sim.cores[t].tensor("logits"))}
            for t in range(NCORES)
        ]

        class R:
            exec_time_ns = None
            instructions_and_trace = None
            profile_json = None

        res = R()
        res.results = results
    else:
        res = run_bass_kernel_spmd(
            _CACHED["nc"], in_maps, core_ids=list(range(NCORES)),
            trace=bool(int(os.environ.get("KERNEL_TRACE", "0"))),
        )
    out = np.zeros((B, T, V1), np.float32)
    for core in range(NCORES):
        b, shard = core // 2, core % 2
        cA, cB = CHUNKS[shard]
        out[b, cA : cA + CH] = res.results[core]["logits"][0:CH]
        out[b, cB : cB + CH] = res.results[core]["logits"][CH:TOK]
    _CACHED["last_result"] = res
    return out


# revision 13
# speedup vs baseline: 1.0293x; 1.0293x over previous
"""GPT (4-layer, B=4 T=1024 C=1024 NH=8) Trainium2 Bass kernel.

Sharding: 8 cores = 4 batches (DP) x 2 balanced-causal sequence shards.
Shard 0 owns token chunks [0,256)+[768,1024); shard 1 owns [256,768).
This balances causal attention work across the shard pair (each core
computes 12 kv-blocks of 128 per head: 4 for its low chunk qA, 8 for its
high chunk qB; invisible blocks are zeroed by data masks so the SPMD
program is identical on every core).

Per layer: LN1 (stats via PE ones-matmuls, rstd via ACT ln/exp, output
written directly in fp8e4 scaled x16), Q/K/V projections as fp8
DoubleRow matmuls (K=256 per instruction), K/V AllGathered (bf16)
between the shard pair as two separate collectives so attention can
start on K while V is still in flight. Gathered K/V stay SBUF-resident.
Scores accumulate 4 blocks per PSUM tile so one ACT exp call evacuates
[128,1024]; causal masks are elementwise only where needed; the
column-disable mask is folded into the softmax denominator matmul
(lhsT = column mask vector) and into zeroed V rows. 1/den comes from
ACT exp(-ln(den)) and is broadcast across partitions with a PE
ones-matmul (no DRAM round trips anywhere). Attention out -> fp8 proj
(DoubleRow) -> residual. LN2 -> MLP in bf16 (fp8 fails the accuracy
budget there) with gelu evacuated straight from PSUM by ACT.
Final LN + per-residue EinLinear head as in the reference.
"""

import os
import sys

import numpy as np

for _p in ("/opt/trn_rl_repo",):
    if _p not in sys.path and os.path.isdir(_p):
        sys.path.insert(0, _p)

import ml_dtypes  # noqa: E402

BF16 = ml_dtypes.bfloat16
F8E4 = ml_dtypes.float8_e4m3  # TRN FP8_EXP4 (max +-240)

# model dims
B, T, C, L = 4, 1024, 1024, 4
NH, HD = 8, 128
HID = 4 * C
V1 = 101  # V + 1
TD = 64  # TOTAL_DIM
NUM_NODE, F_DIM = 15, 4
D_BIAS = NUM_NODE * F_DIM  # 60
EPS = 1e-5
NCORES = 8
TOK = 512  # tokens per core (2 chunks of 256)
CH = 256  # chunk width
KSUB = C // 128  # 8
HSUB = HID // 128  # 32
NL = int(os.environ.get("KERNEL_NLAYERS", str(L)))

# fp8 scales
SA = 16.0  # LN1 output / attention-out scale
SWQ = 8192.0  # wq (after 1/sqrt(hd) fold)
SW = 512.0  # wk/wv/wp
UNS_Q = 1.0 / (SA * SWQ)
UNS_KV = 1.0 / (SA * SW)
LN16 = float(np.log(SA))

# chunk starts per shard: (qA base, qB base)
CHUNKS = {0: (0, 768), 1: (256, 512)}

_CACHED = {}


def _build_program():
    import concourse.bacc as bacc
    import concourse.bass as bass  # noqa: F401
    import concourse.mybir as mybir
    import concourse.tile as tile

    dt = mybir.dt
    AF = mybir.ActivationFunctionType
    OP = mybir.AluOpType
    PM = mybir.MatmulPerfMode

    nc = bacc.Bacc(trn_type="TRN2", num_devices=NCORES)

    # ---- I/O ----
    h0T_d = nc.dram_tensor("h0T", (C, TOK), dt.float32, kind="ExternalInput")
    maskT_d = nc.dram_tensor("maskT", (8, 128, CH), dt.bfloat16, kind="ExternalInput")
    pbT_d = nc.dram_tensor("pbT", (64, NH, 64), dt.float32, kind="ExternalInput")
    wq_d = nc.dram_tensor("wq", (L, C, C), dt.float8e4, kind="ExternalInput")
    wk_d = nc.dram_tensor("wk", (L, C, C), dt.float8e4, kind="ExternalInput")
    wv_d = nc.dram_tensor("wv", (L, C, C), dt.float8e4, kind="ExternalInput")
    wp_d = nc.dram_tensor("wp", (L, C, C), dt.float8e4, kind="ExternalInput")
    w1_d = nc.dram_tensor("w1", (L, C, HID), dt.bfloat16, kind="ExternalInput")
    w2_d = nc.dram_tensor("w2", (L, HID, C), dt.bfloat16, kind="ExternalInput")
    hwT_d = nc.dram_tensor("hwT", (TD, C, V1), dt.bfloat16, kind="ExternalInput")
    cmh_d = nc.dram_tensor("cmh", (128, 8, 8), dt.bfloat16, kind="ExternalInput")
    cm32_d = nc.dram_tensor("cm32", (128, 1), dt.float32, kind="ExternalInput")
    out_d = nc.dram_tensor("logits", (TOK, V1), dt.float32, kind="ExternalOutput")

    RG = [[0, 1], [2, 3], [4, 5], [6, 7]]
    KVHALF = C * TOK  # elements per k (or v) shard

    with tile.TileContext(nc) as tc:
        consts = tc.alloc_tile_pool(name="consts", bufs=1)
        hpool = tc.alloc_tile_pool(name="hpool", bufs=1)
        kvpool = tc.alloc_tile_pool(name="kvpool", bufs=1)
        wpool = tc.alloc_tile_pool(name="wpool", bufs=2)
        w1pool = tc.alloc_tile_pool(name="w1pool", bufs=2)
        w2pool = tc.alloc_tile_pool(name="w2pool", bufs=2)
        apool = tc.alloc_tile_pool(name="apool", bufs=1)
        spool = tc.alloc_tile_pool(name="spool", bufs=2)
        dpool = tc.alloc_tile_pool(name="dpool", bufs=2, space="DRAM")
        ps = tc.alloc_tile_pool(name="ps", bufs=4, space="PSUM")
        psc = tc.alloc_tile_pool(name="psc", bufs=2, space="PSUM")

        # ---- constants ----
        mask_sb = consts.tile([128, 8, CH], dt.bfloat16, name="mask_sb")
        nc.sync.dma_start(mask_sb, maskT_d.rearrange("m p q -> p m q"))
        pb_sb = consts.tile([64, NH, 64], dt.float32, name="pb_sb")
        nc.sync.dma_start(pb_sb, pbT_d[:])
        ones32 = consts.tile([128, 1], dt.float32, name="ones32")
        nc.vector.memset(ones32, 1.0)
        onesrow = consts.tile([1, 128], dt.float32, name="onesrow")
        nc.vector.memset(onesrow, 1.0)
        cmh = consts.tile([128, 8, 8], dt.bfloat16, name="cmh")
        nc.sync.dma_start(cmh, cmh_d[:])
        cm32 = consts.tile([128, 1], dt.float32, name="cm32")
        nc.sync.dma_start(cm32, cm32_d[:])
        ones16 = consts.tile([128, 1], dt.bfloat16, name="ones16")
        nc.vector.memset(ones16, 1.0)
        eps1 = consts.tile([1, 1], dt.float32, name="eps1")
        nc.vector.memset(eps1, EPS)
        bln16 = consts.tile([1, 1], dt.float32, name="bln16")
        nc.vector.memset(bln16, LN16)
        bzero = consts.tile([1, 1], dt.float32, name="bzero")
        nc.vector.memset(bzero, 0.0)

        # ---- residual stream, feature-major fp32 ----
        h_sb = hpool.tile([128, KSUB, TOK], dt.float32, name="h_sb")
        nc.sync.dma_start(h_sb, h0T_d.rearrange("(ko p) q -> p ko q", p=128))

        def layernorm(dst, fp8_scale_bias):
            """LN over C of h_sb -> dst (bf16 or fp8).

            fp8_scale_bias: extra ln-space bias added to rstd (ln(SA) for
            fp8 outputs so dst = SA * normalized, 0.0 for bf16).
            """
            sum_ps = ps.tile([128, TOK], dt.float32, tag="ps", name="sum_ps")
            ssq_ps = ps.tile([128, TOK], dt.float32, tag="ps", name="ssq_ps")
            for ct in range(KSUB):
                nc.tensor.matmul(
                    sum_ps[0:1, :], ones32, h_sb[:, ct, :],
                    start=(ct == 0), stop=(ct == KSUB - 1),
                )
            for ct in range(KSUB):
                sq = spool.tile([128, TOK], dt.bfloat16, tag="sq16", name="sq", bufs=3)
                nc.vector.tensor_mul(sq, h_sb[:, ct, :], h_sb[:, ct, :])
                nc.tensor.matmul(
                    ssq_ps[0:1, :], ones16, sq,
                    start=(ct == 0), stop=(ct == KSUB - 1),
                )
            # stats on one partition: mean, var, then s1 = scale*rstd, s0 = mean*s1
            st = spool.tile([1, 3, TOK], dt.float32, tag="stat", name="st", bufs=1)
            mean = st[:, 0, :]
            var = st[:, 1, :]
            s1 = st[:, 2, :]
            s0 = mean  # overwritten in place below
            nc.vector.tensor_scalar_mul(mean, sum_ps[0:1, :], 1.0 / C)
            nc.vector.tensor_scalar_mul(var, ssq_ps[0:1, :], 1.0 / C)
            msq = spool.tile([1, TOK], dt.float32, tag="msq", name="msq", bufs=1)
            nc.vector.tensor_mul(msq, mean, mean)
            nc.vector.tensor_sub(var, var, msq)
            # rstd = exp(-0.5 * ln(var + eps)) [* SA]
            nc.scalar.activation(var, var, AF.Ln, bias=eps1, scale=1.0)
            nc.scalar.activation(s1, var, AF.Exp, bias=fp8_scale_bias, scale=-0.5)
            nc.vector.tensor_mul(s0, mean, s1)
            # broadcast via PE: statB[p, :] = s for all p
            s1B = ps.tile([128, TOK], dt.float32, tag="ps", name="s1B")
            s0B = ps.tile([128, TOK], dt.float32, tag="ps", name="s0B")
            nc.tensor.matmul(s1B, onesrow, s1, start=True, stop=True)
            nc.tensor.matmul(s0B, onesrow, s0, start=True, stop=True)
            for ct in range(KSUB):
                tmp = spool.tile([128, TOK], dt.float32, tag="scr32", name="lntmp")
                nc.vector.tensor_mul(tmp, h_sb[:, ct, :], s1B)
                nc.vector.tensor_sub(dst[:, ct, :], tmp, s0B)

        for layer in range(NL):
            # ---------- LN1 -> fp8 (x SA) ----------
            aT8 = apool.tile([128, KSUB, TOK], dt.float8e4, tag="a8", name="aT8")
            layernorm(aT8, bln16)

            # ---------- K projection (fp8 DoubleRow) + AllGather ----------
            k_in = dpool.tile([C, TOK], dt.bfloat16, tag="kin", name="k_in")
            k_ga = dpool.tile([2, C, TOK], dt.bfloat16, tag="kga", name="k_ga")
            v_in = dpool.tile([TOK, C], dt.bfloat16, tag="vin", name="v_in")
            v_ga = dpool.tile([2, TOK, C], dt.bfloat16, tag="vga", name="v_ga")

            wk_sb = wpool.tile([128, KSUB, C], dt.float8e4, tag="wmat", name="wk_sb")
            nc.sync.dma_start(wk_sb, wk_d[layer].rearrange("(ko p) n -> p ko n", p=128))
            k_sb = apool.tile([128, NH, TOK], dt.bfloat16, tag="kv_st", name="k_sb")
            for hh in range(NH):
                pk = ps.tile([128, TOK], dt.float32, tag="ps", name="pk")
                for kk in range(KSUB // 2):
                    nc.tensor.matmul(
                        pk,
                        wk_sb[:, 2 * kk : 2 * kk + 2, hh * HD : (hh + 1) * HD],
                        aT8[:, 2 * kk : 2 * kk + 2, :],
                        start=(kk == 0), stop=(kk == KSUB // 2 - 1),
                        perf_mode=PM.DoubleRow,
                    )
                nc.vector.tensor_scalar_mul(k_sb[:, hh, :], pk, UNS_KV)
            nc.gpsimd.dma_start(
                k_in.rearrange("(h d) t -> d h t", d=128), k_sb
            )
            nc.gpsimd.collective_compute(
                "AllGather", OP.bypass, replica_groups=RG,
                ins=[k_in.opt()], outs=[k_ga.opt()],
            )

            # ---------- V projection (fp8 DoubleRow, token-major out) ----------
            wv_sb = wpool.tile([128, KSUB, C], dt.float8e4, tag="wmat", name="wv_sb")
            nc.sync.dma_start(wv_sb, wv_d[layer].rearrange("(ko p) n -> p ko n", p=128))
            v_sb = apool.tile([128, 4, C], dt.bfloat16, tag="kv_st", name="v_sb")
            for tsub in range(4):
                for chalf in range(2):
                    pv = ps.tile([128, 512], dt.float32, tag="ps", name="pv")
                    for kk in range(KSUB // 2):
                        nc.tensor.matmul(
                            pv,
                            aT8[:, 2 * kk : 2 * kk + 2, tsub * 128 : (tsub + 1) * 128],
                            wv_sb[:, 2 * kk : 2 * kk + 2, chalf * 512 : (chalf + 1) * 512],
                            start=(kk == 0), stop=(kk == KSUB // 2 - 1),
                            perf_mode=PM.DoubleRow,
                        )
                    # zero disabled token rows (p%64==63) + unscale
                    nc.vector.tensor_scalar(
                        v_sb[:, tsub, chalf * 512 : (chalf + 1) * 512],
                        pv, cm32, UNS_KV, op0=OP.mult, op1=OP.mult,
                    )
            nc.gpsimd.dma_start(v_in.rearrange("(ts p) c -> p ts c", p=128), v_sb)
            nc.gpsimd.collective_compute(
                "AllGather", OP.bypass, replica_groups=RG,
                ins=[v_in.opt()], outs=[v_ga.opt()],
            )

            # ---------- Q projection (fp8 DoubleRow, overlaps AllGather) ----------
            wq_sb = wpool.tile([128, KSUB, C], dt.float8e4, tag="wmat", name="wq_sb")
            nc.sync.dma_start(wq_sb, wq_d[layer].rearrange("(ko p) n -> p ko n", p=128))
            qT = apool.tile([128, NH, TOK], dt.bfloat16, tag="qT", name="qT")
            for hh in range(NH):
                pq = ps.tile([128, TOK], dt.float32, tag="ps", name="pq")
                for kk in range(KSUB // 2):
                    nc.tensor.matmul(
                        pq,
                        wq_sb[:, 2 * kk : 2 * kk + 2, hh * HD : (hh + 1) * HD],
                        aT8[:, 2 * kk : 2 * kk + 2, :],
                        start=(kk == 0), stop=(kk == KSUB // 2 - 1),
                        perf_mode=PM.DoubleRow,
                    )
                nc.vector.tensor_scalar_mul(qT[:, hh, :], pq, UNS_Q)

            # ---------- gathered K/V -> SBUF (global token order) ----------
            # global chunks: c0 = ga[0][0:256], c1 = ga[1][0:256],
            #                c2 = ga[1][256:512], c3 = ga[0][256:512]
            kg = kvpool.tile([128, NH, T], dt.bfloat16, name="kg")
            CHSRC = [(0, 0), (1, 0), (1, 256), (0, 256)]
            for gch, (r, off) in enumerate(CHSRC):
                nc.gpsimd.dma_start(
                    kg[:, :, gch * CH : (gch + 1) * CH],
                    k_ga[r].rearrange("(h d) t -> d h t", d=128)[
                        :, :, off : off + CH
                    ],
                )
            # vg layout: [p, gblk, h, d]; token = 128*gblk + p
            vg = kvpool.tile([128, 8, NH, HD], dt.bfloat16, name="vg")
            for gch, (r, off) in enumerate(CHSRC):
                nc.gpsimd.dma_start(
                    vg[:, 2 * gch : 2 * gch + 2, :, :],
                    v_ga[r].rearrange("(ts p) (h d) -> p ts h d", p=128, d=128)[
                        :, off // 128 : off // 128 + 2, :, :
                    ],
                )

            # ---------- attention ----------
            # den for all heads lands in one [8, TOK] psum tile (row hh via
            # the one-hot-column mask lhsT, scaled 1/SA); a single DVE
            # reciprocal then yields SA/den for every head at once.
            yT8 = apool.tile([128, NH, TOK], dt.float8e4, tag="y8", name="yT8")
            den_all = ps.tile([8, TOK], dt.float32, tag="ps", name="den_all")
            py_all = spool.tile(
                [128, NH, TOK], dt.bfloat16, tag="py_all", name="py_all", bufs=1
            )
            dstate = {"first": True}
            for hh in range(NH):
                py = ps.tile([128, TOK], dt.float32, tag="ps", name="py")
                pystate = {"first": True}

                def qchunk(qoff, gbase, ngrp, masked, last=False):
                    """One group of up to 4 kv blocks for q cols [qoff,qoff+CH)."""
                    sc = psc.tile([128, 4, CH], dt.float32, tag="sc", name="sc")
                    for i in range(ngrp):
                        g = gbase + i
                        nc.tensor.matmul(
                            sc[:, i, :],
                            kg[:, hh, g * 128 : (g + 1) * 128],
                            qT[:, hh, qoff : qoff + CH],
                            start=True, stop=True,
                        )
                    if gbase == 0 and qoff == 0:
                        # graph bias on (kv<64, q<64); zero data off shard 0
                        nc.vector.tensor_add(
                            sc[0:64, 0, 0:64], sc[0:64, 0, 0:64], pb_sb[:, hh, :]
                        )
                    e = spool.tile([128, 4, CH], dt.bfloat16, tag="e_sb", name="e_sb")
                    nc.scalar.activation(
                        e[:, 0:ngrp, :], sc[:, 0:ngrp, :], AF.Exp
                    )
                    if masked is not None:
                        nc.vector.tensor_mul(
                            e[:, 0:ngrp, :], e[:, 0:ngrp, :],
                            mask_sb[:, masked : masked + ngrp, :],
                        )
                    for i in range(ngrp):
                        g = gbase + i
                        fin = last and (i == ngrp - 1)
                        nc.tensor.matmul(
                            den_all[:, qoff : qoff + CH], cmh[:, hh, :], e[:, i, :],
                            start=dstate["first"], stop=(fin and hh == NH - 1),
                        )
                        dstate["first"] = False
                        nc.tensor.matmul(
                            py[:, qoff : qoff + CH], vg[:, g, hh, :], e[:, i, :],
                            start=pystate["first"], stop=fin,
                        )
                        pystate["first"] = False

                qchunk(0, 0, 4, 0)             # qA: blocks 0..3, masks 0..3
                qchunk(CH, 0, 4, None)         # qB: blocks 0..3, always visible
                qchunk(CH, 4, 4, 4, last=True) # qB: blocks 4..7, masks 4..7
                nc.scalar.copy(py_all[:, hh, :], py)
            rec_sb = spool.tile([8, TOK], dt.float32, tag="rec", name="rec_sb", bufs=1)
            nc.vector.reciprocal(rec_sb, den_all)
            rec_dram = dpool.tile([8, TOK], dt.float32, tag="recd", name="rec_dram")
            nc.scalar.dma_start(rec_dram, rec_sb)
            for hh in range(NH):
                recB = spool.tile([128, TOK], dt.float32, tag="recB", name="recB")
                nc.scalar.dma_start(
                    recB, rec_dram[hh : hh + 1, :].to_broadcast([128, TOK])
                )
                nc.vector.tensor_mul(yT8[:, hh, :], py_all[:, hh, :], recB)

            # ---------- proj (fp8 DoubleRow) + residual ----------
            wp_sb = wpool.tile([128, KSUB, C], dt.float8e4, tag="wmat", name="wp_sb")
            nc.sync.dma_start(wp_sb, wp_d[layer].rearrange("(ko p) n -> p ko n", p=128))
            for co in range(KSUB):
                pp = ps.tile([128, TOK], dt.float32, tag="ps", name="pp")
                for kk in range(KSUB // 2):
                    nc.tensor.matmul(
                        pp,
                        wp_sb[:, 2 * kk : 2 * kk + 2, co * 128 : (co + 1) * 128],
                        yT8[:, 2 * kk : 2 * kk + 2, :],
                        start=(kk == 0), stop=(kk == KSUB // 2 - 1),
                        perf_mode=PM.DoubleRow,
                    )
                pp_sb = spool.tile([128, TOK], dt.float32, tag="scr32", name="pp_sb")
                nc.scalar.mul(pp_sb, pp, UNS_KV)
                nc.vector.tensor_add(h_sb[:, co, :], h_sb[:, co, :], pp_sb)

            # ---------- LN2 -> bf16 ----------
            aT2 = apool.tile([128, KSUB, TOK], dt.bfloat16, tag="a16", name="aT2")
            layernorm(aT2, 0.0)

            # ---------- MLP (bf16) ----------
            g_sb = apool.tile([128, HSUB, TOK], dt.bfloat16, tag="g_sb", name="g_sb")
            for hb in range(8):  # 512 hidden cols at a time
                w1_sb = w1pool.tile([128, KSUB, 512], dt.bfloat16, tag="w1b", name="w1_sb")
                nc.sync.dma_start(
                    w1_sb,
                    w1_d[layer].rearrange("(ko p) n -> p ko n", p=128)[
                        :, :, hb * 512 : (hb + 1) * 512
                    ],
                )
                for hc in range(4):
                    pu = ps.tile([128, TOK], dt.float32, tag="ps", name="pu")
                    for ct in range(KSUB):
                        nc.tensor.matmul(
                            pu,
                            w1_sb[:, ct, hc * 128 : (hc + 1) * 128],
                            aT2[:, ct, :],
                            start=(ct == 0), stop=(ct == KSUB - 1),
                        )
                    nc.scalar.activation(g_sb[:, hb * 4 + hc, :], pu, AF.Gelu)

            # all 8 output c-tiles accumulate at once: 4 ps banks + the 4
            # banks of two idle score tiles; each w2 chunk streams in once.
            pd = [
                ps.tile([128, TOK], dt.float32, tag="ps", name=f"pd{i}")
                for i in range(4)
            ]
            pdx = [
                psc.tile([128, 4, CH], dt.float32, tag="sc", name=f"pdx{i}")
                for i in range(2)
            ]
            pd += [pdx[0][:, 0:2, :], pdx[0][:, 2:4, :],
                   pdx[1][:, 0:2, :], pdx[1][:, 2:4, :]]
            for jc in range(8):  # w2 chunk of 512 hidden rows
                w2_sb = w2pool.tile([128, 4, C], dt.bfloat16, tag="w2t", name="w2_sb")
                nc.sync.dma_start(
                    w2_sb,
                    w2_d[layer][512 * jc : 512 * (jc + 1), :].rearrange(
                        "(ks p) n -> p ks n", p=128
                    ),
                )
                for ks in range(4):
                    ksg = 4 * jc + ks
                    for co in range(8):
                        nc.tensor.matmul(
                            pd[co],
                            w2_sb[:, ks, co * 128 : (co + 1) * 128],
                            g_sb[:, ksg, :],
                            start=(ksg == 0), stop=(ksg == HSUB - 1),
                        )
            for co in range(8):
                nc.vector.tensor_add(h_sb[:, co, :], h_sb[:, co, :], pd[co])

        # ---------- final LN + head ----------
        hfT = apool.tile([128, KSUB, TOK], dt.bfloat16, tag="a16", name="hfT")
        layernorm(hfT, 0.0)
        hfT_r = hfT.rearrange("p k (b e) -> p k e b", e=TD)  # b: 8 blocks of 64
        out_r = out_d.rearrange("(b e) v -> e b v", e=TD)
        for e in range(TD):
            hw_sb = w1pool.tile([128, KSUB, V1], dt.bfloat16, tag="hw", name="hw_sb")
            nc.sync.dma_start(hw_sb, hwT_d[e].rearrange("(ko p) n -> p ko n", p=128))
            po = ps.tile([TOK // TD, V1], dt.float32, tag="ps", name="po")
            for ct in range(KSUB):
                nc.tensor.matmul(
                    po, hfT_r[:, ct, e, :], hw_sb[:, ct, :],
                    start=(ct == 0), stop=(ct == KSUB - 1),
                )
            o_sb = spool.tile([TOK // TD, V1], dt.float32, tag="o_sb", name="o_sb")
            nc.vector.tensor_copy(o_sb, po)
            nc.sync.dma_start(out_r[e], o_sb)

        for p in (psc, ps, dpool, spool, apool, w2pool, w1pool, wpool, kvpool, hpool, consts):
            p.release()

    nc.compile()
    return nc


def _host_inputs(x, attn_bias, pos_emb, Wq, Wk, Wv, Wp, w1, w2, head_w):
    """Build per-core input maps (numpy)."""
    scale = 1.0 / np.sqrt(HD)

    def to8(w, s):
        return np.clip(np.asarray(w, np.float32) * s, -240.0, 240.0).astype(F8E4)

    wq8 = to8(np.asarray(Wq, np.float32) * scale, SWQ)
    wk8 = to8(Wk, SW)
    wv8 = to8(Wv, SW)
    wp8 = to8(Wp, SW)
    w1b = np.asarray(w1, np.float32).astype(BF16)
    w2b = np.asarray(w2, np.float32).astype(BF16)
    hwT = np.ascontiguousarray(
        np.asarray(head_w, np.float32).transpose(0, 2, 1)
    ).astype(BF16)

    # pbias (graph bias) expanded; transposed (kv, head, q), padded 60->64
    bias = np.repeat(np.repeat(np.asarray(attn_bias, np.float32), F_DIM, 1), F_DIM, 2)
    pbT = np.zeros((64, NH, 64), np.float32)
    pbT[:D_BIAS, :, :D_BIAS] = bias.transpose(2, 0, 1)  # [j, h, i]
    pbT_zero = np.zeros_like(pbT)

    h0 = np.asarray(x, np.float32) + np.asarray(pos_emb, np.float32)  # (B, T, C)

    # per-shard causal masks: m in 0..3 -> qA vs kv block m; 4..7 -> qB vs block m
    masks = {}
    for shard, (cA, cB) in CHUNKS.items():
        mk = np.zeros((8, 128, CH), np.float32)
        jq = np.arange(CH)
        for m in range(4):
            kvi = m * 128 + np.arange(128)
            mk[m] = (kvi[:, None] <= (cA + jq)[None, :])
        for m in range(4, 8):
            kvi = m * 128 + np.arange(128)
            mk[m] = (kvi[:, None] <= (cB + jq)[None, :])
        masks[shard] = mk.astype(BF16)

    cmv = np.ones((128, 1), np.float32)
    cmv[63, 0] = 0.0
    cmv[127, 0] = 0.0
    cmh = np.zeros((128, 8, 8), np.float32)
    for _h in range(8):
        cmh[:, _h, _h] = cmv[:, 0] / SA
    cmh = cmh.astype(BF16)
    in_maps = []
    for core in range(NCORES):
        b, shard = core // 2, core % 2
        cA, cB = CHUNKS[shard]
        tok = np.r_[cA : cA + CH, cB : cB + CH]
        h0T = np.ascontiguousarray(h0[b, tok].T)  # (C, TOK)
        in_maps.append(
            {
                "h0T": h0T,
                "maskT": masks[shard],
                "pbT": pbT if shard == 0 else pbT_zero,
                "wq": wq8, "wk": wk8, "wv": wv8, "wp": wp8,
                "w1": w1b, "w2": w2b, "hwT": hwT,
                "cmh": cmh, "cm32": cmv,
            }
        )
    return in_maps


def kernel(**inputs):
    from concourse.bass_utils import run_bass_kernel_spmd

    in_maps = _host_inputs(
        inputs["x"], inputs["attn_bias"], inputs["pos_emb"],
        inputs["Wq"], inputs["Wk"], inputs["Wv"], inputs["Wp"],
        inputs["w1"], inputs["w2"], inputs["head_w"],
    )
    if "nc" not in _CACHED:
        _CACHED["nc"] = _build_program()

    if os.environ.get("KERNEL_SIM", "0") == "1":
        from concourse.bass_interp import MultiCoreSim

        sim = MultiCoreSim(_CACHED["nc"], NCORES)
        for t, m in enumerate(in_maps):
            for k, v in m.items():
                sim.cores[t].tensor(k)[:] = v
        sim.simulate()
        results = [
            {"logits": np.array(sim.cores[t].tensor("logits"))}
            for t in range(NCORES)
        ]

        class R:
            exec_time_ns = None
            instructions_and_trace = None
            profile_json = None

        res = R()
        res.results = results
    else:
        res = run_bass_kernel_spmd(
            _CACHED["nc"], in_maps, core_ids=list(range(NCORES)),
            trace=bool(int(os.environ.get("KERNEL_TRACE", "0"))),
        )
    out = np.zeros((B, T, V1), np.float32)
    for core in range(NCORES):
        b, shard = core // 2, core % 2
        cA, cB = CHUNKS[shard]
        out[b, cA : cA + CH] = res.results[core]["logits"][0:CH]
        out[b, cB : cB + CH] = res.results[core]["logits"][CH:TOK]
    _CACHED["last_result"] = res
    return out


# revision 14
# speedup vs baseline: 1.1179x; 1.0861x over previous
"""GPT (4-layer, B=4 T=1024 C=1024 NH=8) Trainium2 Bass kernel.

Sharding: 8 cores = 4 batches (DP) x 2 balanced-causal sequence shards.
Shard 0 owns token chunks [0,256)+[768,1024); shard 1 owns [256,768).
This balances causal attention work across the shard pair (each core
computes 12 kv-blocks of 128 per head: 4 for its low chunk qA, 8 for its
high chunk qB; invisible blocks are zeroed by data masks so the SPMD
program is identical on every core).

Per layer: LN1 (stats via PE ones-matmuls interleaved into the previous
residual update, rstd via ACT ln/exp, output written directly in fp8e4
scaled x16), Q/K/V projections as fp8 DoubleRow matmuls (K=256 per
instruction), K/V AllGathered (bf16) between the shard pair as two
separate collectives so attention can start on K while V is in flight.
Gathered K/V stay SBUF-resident. Scores accumulate 4 blocks per PSUM
tile so one ACT exp call evacuates [128,1024]; causal masks are
elementwise only where needed; the column-disable mask is folded into
the softmax denominator matmul (one-hot-column lhsT accumulates every
head's den into one psum tile per 4-head group) and into zeroed V rows.
SA/den comes from one DVE reciprocal per group and is broadcast via a
DRAM bounce. Attention out -> fp8 proj (DoubleRow) -> residual.
LN2 -> MLP in bf16 (fp8 fails the accuracy budget there) with gelu
evacuated straight from PSUM by ACT; all 8 w2 output tiles accumulate
in one pass (4 ps banks + 4 borrowed score banks) so each w2 chunk
streams from HBM exactly once. Final LN + per-residue EinLinear head.
Weight DMAs are split between the sync queue (w1/w2 streams) and the
scalar queue (qkv/proj/head) so slot-reuse waits on one stream never
block the other; kv traffic rides the gpsimd queue with the
collectives.
"""

import os
import sys

import numpy as np

for _p in ("/opt/trn_rl_repo",):
    if _p not in sys.path and os.path.isdir(_p):
        sys.path.insert(0, _p)

import ml_dtypes  # noqa: E402

BF16 = ml_dtypes.bfloat16
F8E4 = ml_dtypes.float8_e4m3  # TRN FP8_EXP4 (max +-240)

# model dims
B, T, C, L = 4, 1024, 1024, 4
NH, HD = 8, 128
HID = 4 * C
V1 = 101  # V + 1
TD = 64  # TOTAL_DIM
NUM_NODE, F_DIM = 15, 4
D_BIAS = NUM_NODE * F_DIM  # 60
EPS = 1e-5
NCORES = 8
TOK = 512  # tokens per core (2 chunks of 256)
CH = 256  # chunk width
KSUB = C // 128  # 8
HSUB = HID // 128  # 32
NL = int(os.environ.get("KERNEL_NLAYERS", str(L)))

# fp8 scales
SA = 16.0  # LN1 output / attention-out scale
SWQ = 8192.0  # wq (after 1/sqrt(hd) fold)
SW = 512.0  # wk/wv/wp
UNS_Q = 1.0 / (SA * SWQ)
UNS_KV = 1.0 / (SA * SW)
LN16 = float(np.log(SA))

# chunk starts per shard: (qA base, qB base)
CHUNKS = {0: (0, 768), 1: (256, 512)}

_CACHED = {}


def _build_program():
    import concourse.bacc as bacc
    import concourse.bass as bass  # noqa: F401
    import concourse.mybir as mybir
    import concourse.tile as tile

    dt = mybir.dt
    AF = mybir.ActivationFunctionType
    OP = mybir.AluOpType
    PM = mybir.MatmulPerfMode

    nc = bacc.Bacc(trn_type="TRN2", num_devices=NCORES)

    # ---- I/O ----
    h0T_d = nc.dram_tensor("h0T", (C, TOK), dt.float32, kind="ExternalInput")
    maskT_d = nc.dram_tensor("maskT", (8, 128, CH), dt.bfloat16, kind="ExternalInput")
    pbT_d = nc.dram_tensor("pbT", (64, NH, 64), dt.float32, kind="ExternalInput")
    wq_d = nc.dram_tensor("wq", (L, C, C), dt.float8e4, kind="ExternalInput")
    wk_d = nc.dram_tensor("wk", (L, C, C), dt.float8e4, kind="ExternalInput")
    wv_d = nc.dram_tensor("wv", (L, C, C), dt.float8e4, kind="ExternalInput")
    wp_d = nc.dram_tensor("wp", (L, C, C), dt.float8e4, kind="ExternalInput")
    w1_d = nc.dram_tensor("w1", (L, C, HID), dt.bfloat16, kind="ExternalInput")
    w2_d = nc.dram_tensor("w2", (L, HID, C), dt.bfloat16, kind="ExternalInput")
    hwT_d = nc.dram_tensor("hwT", (TD, C, V1), dt.bfloat16, kind="ExternalInput")
    cmh_d = nc.dram_tensor("cmh", (128, 8, 4), dt.bfloat16, kind="ExternalInput")
    cm32_d = nc.dram_tensor("cm32", (128, 1), dt.float32, kind="ExternalInput")
    out_d = nc.dram_tensor("logits", (TOK, V1), dt.float32, kind="ExternalOutput")

    RG = [[0, 1], [2, 3], [4, 5], [6, 7]]

    with tile.TileContext(nc) as tc:
        consts = tc.alloc_tile_pool(name="consts", bufs=1)
        hpool = tc.alloc_tile_pool(name="hpool", bufs=1)
        kvpool = tc.alloc_tile_pool(name="kvpool", bufs=1)
        wpool = tc.alloc_tile_pool(name="wpool", bufs=2)
        w1pool = tc.alloc_tile_pool(name="w1pool", bufs=2)
        w2pool = tc.alloc_tile_pool(name="w2pool", bufs=2)
        apool = tc.alloc_tile_pool(name="apool", bufs=1)
        spool = tc.alloc_tile_pool(name="spool", bufs=2)
        dpool = tc.alloc_tile_pool(name="dpool", bufs=2, space="DRAM")
        ps = tc.alloc_tile_pool(name="ps", bufs=4, space="PSUM")
        psc = tc.alloc_tile_pool(name="psc", bufs=2, space="PSUM")

        # ---- constants ----
        mask_sb = consts.tile([128, 8, CH], dt.bfloat16, name="mask_sb")
        nc.sync.dma_start(mask_sb, maskT_d.rearrange("m p q -> p m q"))
        pb_sb = consts.tile([64, NH, 64], dt.float32, name="pb_sb")
        nc.sync.dma_start(pb_sb, pbT_d[:])
        ones32 = consts.tile([128, 1], dt.float32, name="ones32")
        nc.vector.memset(ones32, 1.0)
        ones16 = consts.tile([128, 1], dt.bfloat16, name="ones16")
        nc.vector.memset(ones16, 1.0)
        onesrow = consts.tile([1, 128], dt.float32, name="onesrow")
        nc.vector.memset(onesrow, 1.0)
        cmh = consts.tile([128, 8, 4], dt.bfloat16, name="cmh")
        nc.sync.dma_start(cmh, cmh_d[:])
        cm32 = consts.tile([128, 1], dt.float32, name="cm32")
        nc.sync.dma_start(cm32, cm32_d[:])
        eps1 = consts.tile([1, 1], dt.float32, name="eps1")
        nc.vector.memset(eps1, EPS)
        bln16 = consts.tile([1, 1], dt.float32, name="bln16")
        nc.vector.memset(bln16, LN16)

        # ---- residual stream, feature-major fp32 ----
        h_sb = hpool.tile([128, KSUB, TOK], dt.float32, name="h_sb")
        nc.sync.dma_start(h_sb, h0T_d.rearrange("(ko p) q -> p ko q", p=128))

        def ln_stats_tiles():
            sum_ps = ps.tile([128, TOK], dt.float32, tag="ps", name="sum_ps")
            ssq_ps = ps.tile([128, TOK], dt.float32, tag="ps", name="ssq_ps")
            return sum_ps, ssq_ps

        def ln_stats_accum(stats, ct):
            """Accumulate sum/ssq of h_sb[:, ct, :] (call for ct = 0..7)."""
            sum_ps, ssq_ps = stats
            nc.tensor.matmul(
                sum_ps[0:1, :], ones32, h_sb[:, ct, :],
                start=(ct == 0), stop=(ct == KSUB - 1),
            )
            sq = spool.tile([128, TOK], dt.bfloat16, tag="sq16", name="sq", bufs=3)
            nc.vector.tensor_mul(sq, h_sb[:, ct, :], h_sb[:, ct, :])
            nc.tensor.matmul(
                ssq_ps[0:1, :], ones16, sq,
                start=(ct == 0), stop=(ct == KSUB - 1),
            )

        def ln_finalize(stats, dst, fp8_scale_bias):
            """Finish LN from accumulated stats -> dst (bf16 or fp8).

            fp8_scale_bias: ln-space bias on rstd (bln16 for fp8 x SA out,
            0.0 for bf16 out).
            """
            sum_ps, ssq_ps = stats
            st = spool.tile([1, 3, TOK], dt.float32, tag="stat", name="st", bufs=1)
            mean = st[:, 0, :]
            var = st[:, 1, :]
            s1 = st[:, 2, :]
            s0 = mean  # overwritten in place below
            nc.vector.tensor_scalar_mul(mean, sum_ps[0:1, :], 1.0 / C)
            nc.vector.tensor_scalar_mul(var, ssq_ps[0:1, :], 1.0 / C)
            msq = spool.tile([1, TOK], dt.float32, tag="msq", name="msq", bufs=1)
            nc.vector.tensor_mul(msq, mean, mean)
            nc.vector.tensor_sub(var, var, msq)
            # rstd = exp(-0.5 * ln(var + eps)) [* SA]
            nc.scalar.activation(var, var, AF.Ln, bias=eps1, scale=1.0)
            nc.scalar.activation(s1, var, AF.Exp, bias=fp8_scale_bias, scale=-0.5)
            nc.vector.tensor_mul(s0, mean, s1)
            # broadcast via PE: statB[p, :] = s for all p
            s1B = ps.tile([128, TOK], dt.float32, tag="ps", name="s1B")
            s0B = ps.tile([128, TOK], dt.float32, tag="ps", name="s0B")
            nc.tensor.matmul(s1B, onesrow, s1, start=True, stop=True)
            nc.tensor.matmul(s0B, onesrow, s0, start=True, stop=True)
            for ct in range(KSUB):
                tmp = spool.tile([128, TOK], dt.float32, tag="scr32", name="lntmp")
                nc.vector.tensor_mul(tmp, h_sb[:, ct, :], s1B)
                nc.vector.tensor_sub(dst[:, ct, :], tmp, s0B)

        # layer-0 LN1 stats (later layers fold this into the residual adds)
        stats = ln_stats_tiles()
        for ct in range(KSUB):
            ln_stats_accum(stats, ct)

        for layer in range(NL):
            # ---------- LN1 -> fp8 (x SA) ----------
            aT8 = apool.tile([128, KSUB, TOK], dt.float8e4, tag="a8", name="aT8")
            ln_finalize(stats, aT8, bln16)

            # ---------- K projection (fp8 DoubleRow) + AllGather ----------
            k_in = dpool.tile([C, TOK], dt.bfloat16, tag="kin", name="k_in")
            k_ga = dpool.tile([2, C, TOK], dt.bfloat16, tag="kga", name="k_ga")
            v_in = dpool.tile([TOK, C], dt.bfloat16, tag="vin", name="v_in")
            v_ga = dpool.tile([2, TOK, C], dt.bfloat16, tag="vga", name="v_ga")

            wk_sb = wpool.tile([128, KSUB, C], dt.float8e4, tag="wmat", name="wk_sb")
            nc.scalar.dma_start(wk_sb, wk_d[layer].rearrange("(ko p) n -> p ko n", p=128))
            k_sb = apool.tile([128, NH, TOK], dt.bfloat16, tag="kv_st", name="k_sb")
            for hh in range(NH):
                pk = ps.tile([128, TOK], dt.float32, tag="ps", name="pk")
                for kk in range(KSUB // 2):
                    nc.tensor.matmul(
                        pk,
                        wk_sb[:, 2 * kk : 2 * kk + 2, hh * HD : (hh + 1) * HD],
                        aT8[:, 2 * kk : 2 * kk + 2, :],
                        start=(kk == 0), stop=(kk == KSUB // 2 - 1),
                        perf_mode=PM.DoubleRow,
                    )
                nc.vector.tensor_scalar_mul(k_sb[:, hh, :], pk, UNS_KV)
            nc.gpsimd.dma_start(
                k_in.rearrange("(h d) t -> d h t", d=128), k_sb
            )
            nc.gpsimd.collective_compute(
                "AllGather", OP.bypass, replica_groups=RG,
                ins=[k_in.opt()], outs=[k_ga.opt()],
            )

            # ---------- V projection (fp8 DoubleRow, token-major out) ----------
            wv_sb = wpool.tile([128, KSUB, C], dt.float8e4, tag="wmat", name="wv_sb")
            nc.scalar.dma_start(wv_sb, wv_d[layer].rearrange("(ko p) n -> p ko n", p=128))
            v_sb = apool.tile([128, 4, C], dt.bfloat16, tag="kv_st", name="v_sb")
            for tsub in range(4):
                for chalf in range(2):
                    pv = ps.tile([128, 512], dt.float32, tag="ps", name="pv")
                    for kk in range(KSUB // 2):
                        nc.tensor.matmul(
                            pv,
                            aT8[:, 2 * kk : 2 * kk + 2, tsub * 128 : (tsub + 1) * 128],
                            wv_sb[:, 2 * kk : 2 * kk + 2, chalf * 512 : (chalf + 1) * 512],
                            start=(kk == 0), stop=(kk == KSUB // 2 - 1),
                            perf_mode=PM.DoubleRow,
                        )
                    # zero disabled token rows (p%64==63) + unscale
                    nc.vector.tensor_scalar(
                        v_sb[:, tsub, chalf * 512 : (chalf + 1) * 512],
                        pv, cm32, UNS_KV, op0=OP.mult, op1=OP.mult,
                    )
            nc.gpsimd.dma_start(v_in.rearrange("(ts p) c -> p ts c", p=128), v_sb)
            nc.gpsimd.collective_compute(
                "AllGather", OP.bypass, replica_groups=RG,
                ins=[v_in.opt()], outs=[v_ga.opt()],
            )

            # ---------- Q projection (fp8 DoubleRow, overlaps AllGather) ----------
            wq_sb = wpool.tile([128, KSUB, C], dt.float8e4, tag="wmat", name="wq_sb")
            nc.scalar.dma_start(wq_sb, wq_d[layer].rearrange("(ko p) n -> p ko n", p=128))
            qT = apool.tile([128, NH, TOK], dt.bfloat16, tag="qT", name="qT")
            for hh in range(NH):
                pq = ps.tile([128, TOK], dt.float32, tag="ps", name="pq")
                for kk in range(KSUB // 2):
                    nc.tensor.matmul(
                        pq,
                        wq_sb[:, 2 * kk : 2 * kk + 2, hh * HD : (hh + 1) * HD],
                        aT8[:, 2 * kk : 2 * kk + 2, :],
                        start=(kk == 0), stop=(kk == KSUB // 2 - 1),
                        perf_mode=PM.DoubleRow,
                    )
                nc.vector.tensor_scalar_mul(qT[:, hh, :], pq, UNS_Q)

            # ---------- gathered K/V -> SBUF (global token order) ----------
            # global chunks: c0 = ga[0][0:256], c1 = ga[1][0:256],
            #                c2 = ga[1][256:512], c3 = ga[0][256:512]
            kg = kvpool.tile([128, NH, T], dt.bfloat16, name="kg")
            CHSRC = [(0, 0), (1, 0), (1, 256), (0, 256)]
            for gch, (r, off) in enumerate(CHSRC):
                nc.gpsimd.dma_start(
                    kg[:, :, gch * CH : (gch + 1) * CH],
                    k_ga[r].rearrange("(h d) t -> d h t", d=128)[
                        :, :, off : off + CH
                    ],
                )
            # vg layout: [p, gblk, h, d]; token = 128*gblk + p
            vg = kvpool.tile([128, 8, NH, HD], dt.bfloat16, name="vg")
            for gch, (r, off) in enumerate(CHSRC):
                nc.gpsimd.dma_start(
                    vg[:, 2 * gch : 2 * gch + 2, :, :],
                    v_ga[r].rearrange("(ts p) (h d) -> p ts h d", p=128, d=128)[
                        :, off // 128 : off // 128 + 2, :, :
                    ],
                )

            # ---------- attention (two 4-head groups) ----------
            # per group the 4 heads' denominators land in one [4, TOK] psum
            # tile (one-hot-column lhsT, scaled 1/SA); one DVE reciprocal
            # yields SA/den for the group, broadcast via a DRAM bounce.
            yT8 = apool.tile([128, NH, TOK], dt.float8e4, tag="y8", name="yT8")
            for hg in range(2):
                den_g = ps.tile([4, TOK], dt.float32, tag="ps", name="den_g")
                py_g = spool.tile(
                    [128, 4, TOK], dt.bfloat16, tag="py_g", name="py_g", bufs=2
                )
                dstate = {"first": True}
                for hl in range(4):
                    hh = 4 * hg + hl
                    py = ps.tile([128, TOK], dt.float32, tag="ps", name="py")
                    pystate = {"first": True}

                    def qchunk(qoff, gbase, ngrp, masked, last=False):
                        """Up to 4 kv blocks for q cols [qoff, qoff+CH)."""
                        sc = psc.tile([128, 4, CH], dt.float32, tag="sc", name="sc")
                        for i in range(ngrp):
                            g = gbase + i
                            nc.tensor.matmul(
                                sc[:, i, :],
                                kg[:, hh, g * 128 : (g + 1) * 128],
                                qT[:, hh, qoff : qoff + CH],
                                start=True, stop=True,
                            )
                        if gbase == 0 and qoff == 0:
                            # graph bias (kv<64, q<64); zero data off shard 0
                            nc.vector.tensor_add(
                                sc[0:64, 0, 0:64], sc[0:64, 0, 0:64], pb_sb[:, hh, :]
                            )
                        e = spool.tile([128, 4, CH], dt.bfloat16, tag="e_sb", name="e_sb")
                        nc.scalar.activation(
                            e[:, 0:ngrp, :], sc[:, 0:ngrp, :], AF.Exp
                        )
                        if masked is not None:
                            nc.vector.tensor_mul(
                                e[:, 0:ngrp, :], e[:, 0:ngrp, :],
                                mask_sb[:, masked : masked + ngrp, :],
                            )
                        for i in range(ngrp):
                            g = gbase + i
                            fin = last and (i == ngrp - 1)
                            nc.tensor.matmul(
                                den_g[:, qoff : qoff + CH], cmh[:, hh, :], e[:, i, :],
                                start=dstate["first"], stop=(fin and hl == 3),
                            )
                            dstate["first"] = False
                            nc.tensor.matmul(
                                py[:, qoff : qoff + CH], vg[:, g, hh, :], e[:, i, :],
                                start=pystate["first"], stop=fin,
                            )
                            pystate["first"] = False

                    qchunk(0, 0, 4, 0)              # qA: blocks 0..3, masks 0..3
                    qchunk(CH, 0, 4, None)          # qB: blocks 0..3, full
                    qchunk(CH, 4, 4, 4, last=True)  # qB: blocks 4..7, masks 4..7
                    nc.scalar.copy(py_g[:, hl, :], py)
                rec_sb = spool.tile([4, TOK], dt.float32, tag="rec", name="rec_sb")
                nc.vector.reciprocal(rec_sb, den_g)
                rec_dram = dpool.tile([4, TOK], dt.float32, tag="recd", name="rec_dram")
                nc.scalar.dma_start(rec_dram, rec_sb)
                for hl in range(4):
                    recB = spool.tile([128, TOK], dt.float32, tag="recB", name="recB")
                    nc.scalar.dma_start(
                        recB, rec_dram[hl : hl + 1, :].to_broadcast([128, TOK])
                    )
                    nc.vector.tensor_mul(
                        yT8[:, 4 * hg + hl, :], py_g[:, hl, :], recB
                    )

            # ---------- proj (fp8 DoubleRow) + residual + LN2 stats ----------
            wp_sb = wpool.tile([128, KSUB, C], dt.float8e4, tag="wmat", name="wp_sb")
            nc.scalar.dma_start(wp_sb, wp_d[layer].rearrange("(ko p) n -> p ko n", p=128))
            stats = ln_stats_tiles()
            for co in range(KSUB):
                pp = ps.tile([128, TOK], dt.float32, tag="ps", name="pp")
                for kk in range(KSUB // 2):
                    nc.tensor.matmul(
                        pp,
                        wp_sb[:, 2 * kk : 2 * kk + 2, co * 128 : (co + 1) * 128],
                        yT8[:, 2 * kk : 2 * kk + 2, :],
                        start=(kk == 0), stop=(kk == KSUB // 2 - 1),
                        perf_mode=PM.DoubleRow,
                    )
                pp_sb = spool.tile([128, TOK], dt.float32, tag="scr32", name="pp_sb")
                nc.scalar.mul(pp_sb, pp, UNS_KV)
                nc.vector.tensor_add(h_sb[:, co, :], h_sb[:, co, :], pp_sb)
                ln_stats_accum(stats, co)

            # ---------- LN2 -> bf16 ----------
            aT2 = apool.tile([128, KSUB, TOK], dt.bfloat16, tag="a16", name="aT2")
            ln_finalize(stats, aT2, 0.0)

            # ---------- MLP (bf16) ----------
            g_sb = apool.tile([128, HSUB, TOK], dt.bfloat16, tag="g_sb", name="g_sb")
            for hb in range(8):  # 512 hidden cols at a time
                w1_sb = w1pool.tile([128, KSUB, 512], dt.bfloat16, tag="w1b", name="w1_sb")
                nc.sync.dma_start(
                    w1_sb,
                    w1_d[layer].rearrange("(ko p) n -> p ko n", p=128)[
                        :, :, hb * 512 : (hb + 1) * 512
                    ],
                )
                for hc in range(4):
                    pu = ps.tile([128, TOK], dt.float32, tag="ps", name="pu")
                    for ct in range(KSUB):
                        nc.tensor.matmul(
                            pu,
                            w1_sb[:, ct, hc * 128 : (hc + 1) * 128],
                            aT2[:, ct, :],
                            start=(ct == 0), stop=(ct == KSUB - 1),
                        )
                    nc.scalar.activation(g_sb[:, hb * 4 + hc, :], pu, AF.Gelu)

            # all 8 output c-tiles accumulate at once: 4 ps banks + the 4
            # banks of two idle score tiles; each w2 chunk streams in once.
            pd = [
                ps.tile([128, TOK], dt.float32, tag="ps", name=f"pd{i}")
                for i in range(4)
            ]
            pdx = [
                psc.tile([128, 4, CH], dt.float32, tag="sc", name=f"pdx{i}")
                for i in range(2)
            ]
            pd += [pdx[0][:, 0:2, :], pdx[0][:, 2:4, :],
                   pdx[1][:, 0:2, :], pdx[1][:, 2:4, :]]
            for jc in range(8):  # w2 chunk of 512 hidden rows
                w2_sb = w2pool.tile([128, 4, C], dt.bfloat16, tag="w2t", name="w2_sb")
                nc.sync.dma_start(
                    w2_sb,
                    w2_d[layer][512 * jc : 512 * (jc + 1), :].rearrange(
                        "(ks p) n -> p ks n", p=128
                    ),
                )
                for ks in range(4):
                    ksg = 4 * jc + ks
                    for co in range(8):
                        nc.tensor.matmul(
                            pd[co],
                            w2_sb[:, ks, co * 128 : (co + 1) * 128],
                            g_sb[:, ksg, :],
                            start=(ksg == 0), stop=(ksg == HSUB - 1),
                        )
            stats = ln_stats_tiles()
            for co in range(8):
                nc.vector.tensor_add(h_sb[:, co, :], h_sb[:, co, :], pd[co])
                ln_stats_accum(stats, co)

        # ---------- final LN + head ----------
        hfT = apool.tile([128, KSUB, TOK], dt.bfloat16, tag="a16", name="hfT")
        ln_finalize(stats, hfT, 0.0)
        hfT_r = hfT.rearrange("p k (b e) -> p k e b", e=TD)  # b: 8 blocks of 64
        out_r = out_d.rearrange("(b e) v -> e b v", e=TD)
        for e in range(TD):
            hw_sb = w1pool.tile([128, KSUB, V1], dt.bfloat16, tag="hw", name="hw_sb")
            nc.scalar.dma_start(hw_sb, hwT_d[e].rearrange("(ko p) n -> p ko n", p=128))
            po = ps.tile([TOK // TD, V1], dt.float32, tag="ps", name="po")
            for ct in range(KSUB):
                nc.tensor.matmul(
                    po, hfT_r[:, ct, e, :], hw_sb[:, ct, :],
                    start=(ct == 0), stop=(ct == KSUB - 1),
                )
            o_sb = spool.tile([TOK // TD, V1], dt.float32, tag="o_sb", name="o_sb")
            nc.vector.tensor_copy(o_sb, po)
            nc.sync.dma_start(out_r[e], o_sb)

        for p in (psc, ps, dpool, spool, apool, w2pool, w1pool, wpool, kvpool, hpool, consts):
            p.release()

    nc.compile()
    return nc


def _host_inputs(x, attn_bias, pos_emb, Wq, Wk, Wv, Wp, w1, w2, head_w):
    """Build per-core input maps (numpy)."""
    scale = 1.0 / np.sqrt(HD)

    def to8(w, s):
        return np.clip(np.asarray(w, np.float32) * s, -240.0, 240.0).astype(F8E4)

    wq8 = to8(np.asarray(Wq, np.float32) * scale, SWQ)
    wk8 = to8(Wk, SW)
    wv8 = to8(Wv, SW)
    wp8 = to8(Wp, SW)
    w1b = np.asarray(w1, np.float32).astype(BF16)
    w2b = np.asarray(w2, np.float32).astype(BF16)
    hwT = np.ascontiguousarray(
        np.asarray(head_w, np.float32).transpose(0, 2, 1)
    ).astype(BF16)

    # pbias (graph bias) expanded; transposed (kv, head, q), padded 60->64
    bias = np.repeat(np.repeat(np.asarray(attn_bias, np.float32), F_DIM, 1), F_DIM, 2)
    pbT = np.zeros((64, NH, 64), np.float32)
    pbT[:D_BIAS, :, :D_BIAS] = bias.transpose(2, 0, 1)  # [j, h, i]
    pbT_zero = np.zeros_like(pbT)

    h0 = np.asarray(x, np.float32) + np.asarray(pos_emb, np.float32)  # (B, T, C)

    # per-shard causal masks: m in 0..3 -> qA vs kv block m; 4..7 -> qB vs block m
    masks = {}
    for shard, (cA, cB) in CHUNKS.items():
        mk = np.zeros((8, 128, CH), np.float32)
        jq = np.arange(CH)
        for m in range(4):
            kvi = m * 128 + np.arange(128)
            mk[m] = (kvi[:, None] <= (cA + jq)[None, :])
        for m in range(4, 8):
            kvi = m * 128 + np.arange(128)
            mk[m] = (kvi[:, None] <= (cB + jq)[None, :])
        masks[shard] = mk.astype(BF16)

    cmv = np.ones((128, 1), np.float32)
    cmv[63, 0] = 0.0
    cmv[127, 0] = 0.0
    cmh = np.zeros((128, 8, 4), np.float32)
    for _h in range(8):
        cmh[:, _h, _h % 4] = cmv[:, 0] / SA
    cmh = cmh.astype(BF16)
    in_maps = []
    for core in range(NCORES):
        b, shard = core // 2, core % 2
        cA, cB = CHUNKS[shard]
        tok = np.r_[cA : cA + CH, cB : cB + CH]
        h0T = np.ascontiguousarray(h0[b, tok].T)  # (C, TOK)
        in_maps.append(
            {
                "h0T": h0T,
                "maskT": masks[shard],
                "pbT": pbT if shard == 0 else pbT_zero,
                "wq": wq8, "wk": wk8, "wv": wv8, "wp": wp8,
                "w1": w1b, "w2": w2b, "hwT": hwT,
                "cmh": cmh, "cm32": cmv,
            }
        )
    return in_maps


def kernel(**inputs):
    from concourse.bass_utils import run_bass_kernel_spmd

    in_maps = _host_inputs(
        inputs["x"], inputs["attn_bias"], inputs["pos_emb"],
        inputs["Wq"], inputs["Wk"], inputs["Wv"], inputs["Wp"],
        inputs["w1"], inputs["w2"], inputs["head_w"],
    )
    if "nc" not in _CACHED:
        _CACHED["nc"] = _build_program()

    if os.environ.get("KERNEL_SIM", "0") == "1":
        from concourse.bass_interp import MultiCoreSim

        sim = MultiCoreSim(_CACHED["nc"], NCORES)
        for t, m in enumerate(in_maps):
            for k, v in m.items():
                sim.cores[t].tensor(k)[:] = v
        sim.simulate()
        results = [
            {"logits": np.array(sim.cores[t].tensor("logits"))}
            for t in range(NCORES)
        ]

        class R:
            exec_time_ns = None
            instructions_and_trace = None
            profile_json = None

        res = R()
        res.results = results
    else:
        res = run_bass_kernel_spmd(
            _CACHED["nc"], in_maps, core_ids=list(range(NCORES)),
            trace=bool(int(os.environ.get("KERNEL_TRACE", "0"))),
        )
    out = np.zeros((B, T, V1), np.float32)
    for core in range(NCORES):
        b, shard = core // 2, core % 2
        cA, cB = CHUNKS[shard]
        out[b, cA : cA + CH] = res.results[core]["logits"][0:CH]
        out[b, cB : cB + CH] = res.results[core]["logits"][CH:TOK]
    _CACHED["last_result"] = res
    return out
